# revision 49
# baseline (speedup 1.0000x reference)
"""HF OpenMoe attention (B=2,S=2048,HID=2048,NH=16,NKV=4,HD=128) on 8 trn2 cores.

Sharding: core c -> (batch b=c//4, kv-group g=c%4). Each core computes Q/K/V
projections for its 4 query heads + 1 kv head, RoPE, causal flash attention in
S^T layout (scores transposed: [k, q], softmax over the partition dim via
ones-matmul), and its partial o_proj; a 4-way ReduceScatter sums the o_proj
partials, each core returning a 512-row slice of o^T for its batch.

Phase A streams x^T per token block with weights interleaved into the DMA
stream (wk chunked between xt tiles, wv mid-stream, wq after block 0), PE
chains emitted as operands land, rope on DVE/Act/Pool. Phase B runs causal
attention per q-block in two 2-head passes: score pairs land in a 2-bank PSUM
tile, one exp covers both heads, mask/acc run as paired 2x DVE ops, the
paired denominator reuses the score-PSUM ring, and o_proj of the previous
block is emitted in 1-co units between score tiles so the PE keeps running
through the Act-paced exp stream. The final block (j=0, shortest) holds a few
units in reserve to cover its normalize latency; tail o_proj DMAs are split
small and output DMAs ride the Act queue.
"""
import numpy as np
import concourse.bass as bass
import concourse.bacc as bacc
import concourse.tile as tile
import concourse.mybir as mybir
from concourse.bass_utils import run_bass_kernel_spmd

f32 = mybir.dt.float32
f32r = mybir.dt.float32r
bf16 = mybir.dt.bfloat16
AF = mybir.ActivationFunctionType
MUL = mybir.AluOpType.mult
ADD = mybir.AluOpType.add

B, S, HID = 2, 2048, 2048
NH, NKV, HD = 16, 4, 128
GH = NH // NKV          # query heads per core (4)
TB = 512                # token block (q block / projection block)
NT = S // TB            # 4 token blocks
NCT = HID // 128        # 16 contraction tiles
NKT = S // 128          # 16 key tiles

_CACHE = {}


def _build(causal: bool, with_rs: bool = True):
    nc = bacc.Bacc("TRN2", target_bir_lowering=False, debug=False, num_devices=8)
    xt = nc.dram_tensor("xt", [HID, S], f32, kind="ExternalInput").ap()
    wq = nc.dram_tensor("wq", [HID, GH * HD], f32, kind="ExternalInput").ap()
    wk = nc.dram_tensor("wk", [HID, HD], f32, kind="ExternalInput").ap()
    wv = nc.dram_tensor("wv", [HID, HD], f32, kind="ExternalInput").ap()
    wo = nc.dram_tensor("wo", [GH * HD, HID], f32, kind="ExternalInput").ap()
    cos_d = nc.dram_tensor("cos_t", [HD, S], f32, kind="ExternalInput").ap()
    sin_d = nc.dram_tensor("sin_m", [HD, S], f32, kind="ExternalInput").ap()
    cm_d = nc.dram_tensor("cmask", [128, 1536], bf16, kind="ExternalInput").ap()
    on_d = nc.dram_tensor("ones_in", [128, 128], bf16, kind="ExternalInput").ap()
    id_d = nc.dram_tensor("ident_in", [128, 128], bf16, kind="ExternalInput").ap()
    out_r = nc.dram_tensor("out_r", [TB, S], f32, kind="ExternalOutput").ap()

    with tile.TileContext(nc) as tc:
        with (
            tc.tile_pool(name="glob", bufs=1) as glob,
            tc.tile_pool(name="dram", bufs=1, space="DRAM") as dram,
        ):
            # ---- global resident stores ----
            kt_rope = glob.tile([128, S], f32r, tag="kt")          # roped K^T [d, k]
            v_all = glob.tile([128, S], bf16, tag="v")             # V natural (bf16)
            qt_rope = [glob.tile([128, S], f32r, tag=f"q{h}", name=f"qt_rope{h}")
                       for h in range(GH)]
            cm_b = glob.tile([128, 1536], bf16, tag="cmb")         # paired 0/1 masks
            ones_b = glob.tile([128, 128], bf16, tag="onesb")
            ident_b = glob.tile([128, 128], bf16, tag="identb")

            oT_part = dram.tile([HID, S], f32)                     # o^T partial
            oT_red = dram.tile([TB, S], f32)

            # ---- phase A: projections + rope (phase-scoped SBUF) ----
            with tc.tile_pool(name="pA", bufs=1) as pA, \
                 tc.tile_pool(name="psA", bufs=1, space="PSUM") as psA:
                # batched weight loads: DRAM [c, d] -> SBUF [c-sub(128), ci, d].
                # Issue order matters: the DMA engines drain in order, so wk
                # comes first (chunked between xt tiles), wv/wq mid-stream,
                # and wo not until phase B.
                wk_all = pA.tile([128, NCT, HD], f32r, tag="wk")
                nc.sync.dma_start(wk_all[:, 0:1, :], wk[0:128, :].bitcast(f32r)
                                  .rearrange("(c p) d -> p c d", p=128))
                wv_all = pA.tile([128, NCT, HD], f32r, tag="wv")
                wq_all = [pA.tile([128, NCT, HD], f32r, tag=f"wqh{h}",
                                  name=f"wq_all{h}") for h in range(GH)]

                def rope(ps, dst_ap, cs, sn):
                    """dst = ps*cos + swap64(ps)*sin_mod for token block tb."""
                    raw = pA.tile([128, TB], f32, tag="raw", bufs=3, name="raw")
                    nc.scalar.copy(raw[:], ps[:])
                    rot = pA.tile([128, TB], f32, tag="rot", bufs=6, name="rot")
                    nc.gpsimd.dma_start(rot[0:64, :], raw[64:128, :])
                    nc.gpsimd.dma_start(rot[64:128, :], raw[0:64, :])
                    m1 = pA.tile([128, TB], f32, tag="m1", bufs=6, name="m1")
                    nc.vector.tensor_tensor(m1[:], ps[:], cs[:], op=MUL)  # PSUM: DVE
                    nc.vector.tensor_tensor(rot[:], rot[:], sn[:], op=MUL)
                    nc.vector.tensor_tensor(dst_ap, m1[:], rot[:], op=ADD)

                for tb in range(NT):
                    cos_s = pA.tile([128, TB], f32, tag="cos", bufs=4, name="cos")
                    sin_s = pA.tile([128, TB], f32, tag="sin", bufs=4, name="sin")
                    if tb != 0:
                        # rope tables just ahead of the xt tiles
                        nc.sync.dma_start(cos_s[:], cos_d[:, TB * tb:TB * (tb + 1)])
                        nc.sync.dma_start(sin_s[:], sin_d[:, TB * tb:TB * (tb + 1)])
                    xt_t = []
                    for ci in range(NCT):
                        t = pA.tile([128, TB], f32r, tag="xt", bufs=31, name="xt")
                        nc.sync.dma_start(
                            t[:], xt[128 * ci:128 * (ci + 1),
                                     TB * tb:TB * (tb + 1)].bitcast(f32r))
                        xt_t.append(t)
                        if tb == 0 and ci == 0:
                            nc.sync.dma_start(
                                wk_all[:, 1:4, :], wk[128:512, :].bitcast(f32r)
                                .rearrange("(c p) d -> p c d", p=128))
                        if tb == 0 and ci % 4 == 3 and ci < 15:
                            c = ci // 4 + 1  # stream wk in behind the xt tiles
                            nc.sync.dma_start(
                                wk_all[:, 4 * c:4 * (c + 1), :],
                                wk[512 * c:512 * (c + 1), :].bitcast(f32r)
                                .rearrange("(c p) d -> p c d", p=128))
                        if tb == 0 and ci == 0:
                            nc.sync.dma_start(cos_s[:], cos_d[:, 0:TB])
                            nc.sync.dma_start(sin_s[:], sin_d[:, 0:TB])
                        if tb == 0 and ci == 1:
                            nc.sync.dma_start(ident_b[:], id_d[:])
                            warm = pA.tile([128, 1], f32, tag="warm")
                            nc.scalar.activation(warm[:], ident_b[:, 0:1], AF.Exp)
                        if tb == 0 and ci == 6:
                            # wv early: V proj fills the K-chain's DMA lag
                            nc.sync.dma_start(wv_all[:], wv[:].bitcast(f32r)
                                              .rearrange("(c p) d -> p c d", p=128))
                        if tb == 0 and ci == 10:
                            # first q head's weights ahead of the rest
                            nc.sync.dma_start(
                                wq_all[0][:], wq[:, 0:HD]
                                .bitcast(f32r).rearrange("(c p) d -> p c d", p=128))
                    if tb == 0:
                        for h in range(1, GH):
                            nc.sync.dma_start(
                                wq_all[h][:], wq[:, HD * h:HD * (h + 1)]
                                .bitcast(f32r).rearrange("(c p) d -> p c d", p=128))
                    # K
                    ps_k = psA.tile([128, TB], f32, tag="pk")
                    for ci in range(NCT):
                        nc.tensor.matmul(ps_k[:], wk_all[:, ci, :], xt_t[ci][:],
                                         start=ci == 0, stop=ci == NCT - 1)
                    rope(ps_k, kt_rope[:, TB * tb:TB * (tb + 1)], cos_s, sin_s)

                    def emit_v():
                        ps_v = psA.tile([128, TB], f32, tag="pv")
                        for ci in range(NCT):
                            nc.tensor.matmul(ps_v[:], wv_all[:, ci, :],
                                             xt_t[ci][:],
                                             start=ci == 0, stop=ci == NCT - 1)
                        vt_sb = pA.tile([128, TB], bf16, tag="vts", bufs=3,
                                        name="vt_sb")
                        nc.scalar.copy(vt_sb[:], ps_v[:])
                        return vt_sb

                    def emit_vtrans(vt_sb, u):
                        ps_tr = psA.tile([128, 128], bf16, tag="ptr", bufs=1,
                                         name="ps_tr")
                        nc.tensor.transpose(ps_tr[:],
                                            vt_sb[:, 128 * u:128 * (u + 1)],
                                            ident_b[:])
                        # Act (not DVE): DVE's in-order queue sits behind rope
                        # m1 ops that can wait on the cos/sin loads
                        nc.scalar.copy(
                            v_all[:, 128 * (4 * tb + u):128 * (4 * tb + u + 1)],
                            ps_tr[:])

                    vt_sb = emit_v() if tb < NT - 1 else None
                    # Q heads; one V transpose is spread between each pair of
                    # head blocks so the single ptr bank's WAR (on the previous
                    # transpose's drain copy) never stalls the PE
                    for h in range(GH):
                        ps_q = psA.tile([128, TB], f32, tag="pq", bufs=5,
                                        name=f"ps_q{h}")
                        for ci in range(NCT):
                            nc.tensor.matmul(ps_q[:], wq_all[h][:, ci, :],
                                             xt_t[ci][:],
                                             start=ci == 0, stop=ci == NCT - 1)
                        if vt_sb is not None:
                            emit_vtrans(vt_sb, h)
                        rope(ps_q, qt_rope[h][:, TB * tb:TB * (tb + 1)], cos_s,
                             sin_s)
                    if vt_sb is None:
                        # last block: V after the Q heads, hiding the final
                        # rope chain's latency behind V's matmuls
                        vt_sb = emit_v()
                        for u in range(4):
                            emit_vtrans(vt_sb, u)

            # ---- phase B: attention (2-head passes) + partial o_proj ----
            with tc.tile_pool(name="pB", bufs=1) as pB, \
                 tc.tile_pool(name="psB", bufs=1, space="PSUM") as psB:
                # small constants are bf16 in DRAM: direct loads, no casts
                nc.sync.dma_start(cm_b[:], cm_d[:])
                nc.sync.dma_start(ones_b[:], on_d[:])
                # o_proj weights: first o_proj unit runs well into phase B, so
                # this load hides behind the first attention block
                wo_all = pB.tile([128, GH * HID], f32r, tag="wo")  # [j-sub, jh*2048+c]
                nc.sync.dma_start(wo_all[:].rearrange("p (h c) -> p h c", h=GH),
                                  wo[:].bitcast(f32r)
                                  .rearrange("(h p) c -> p h c", p=128))

                units = []   # pending o_proj 1-co unit closures

                def oproj_unit(j, at_j, co, dma_n, drain=False):
                    """One 128-col chunk of q-block j's o_proj. dma_n=2: flush
                    the 2-co ob group; dma_n=1: single-co DMA. In drain mode
                    (no tile stream running) copies favor Act and DMAs spread
                    over the idle sync/gpsimd queues."""
                    gi = co % 2
                    ob = oproj_unit.ob
                    if gi == 0 and dma_n != 1:
                        ob = oproj_unit.ob = pB.tile([128, 2, TB], f32,
                                                     tag="ob", bufs=4,
                                                     name="ob")
                    ps_p = psB.tile([128, TB], f32, tag="ps_d", bufs=2,
                                    name="ps_p")
                    for jh in range(GH):
                        nc.tensor.matmul(ps_p[:],
                                         wo_all[:, jh * HID + 128 * co:
                                                jh * HID + 128 * (co + 1)],
                                         at_j[jh][:], start=(jh == 0),
                                         stop=(jh == GH - 1))
                    if dma_n == 1:
                        if co < 4 and not with_rs:
                            dst = out_r[128 * co:128 * (co + 1),
                                        TB * j:TB * (j + 1)]
                            obt = pB.tile([128, TB], f32, tag="obt", bufs=2,
                                          name="obt")
                        else:
                            dst = oT_part[128 * co:128 * (co + 1),
                                          TB * j:TB * (j + 1)]
                            obt = pB.tile([128, TB], f32, tag="obtb", bufs=2,
                                          name="obtb")
                        if co % 2:
                            nc.scalar.copy(obt[:], ps_p[:])
                            # keep the Act queue free of DMA issues ahead of
                            # the final copy: only the very last co rides Act
                            q = nc.scalar if co == NCT - 1 else nc.sync
                            q.dma_start(dst, obt[:])
                        else:
                            nc.vector.tensor_copy(obt[:], ps_p[:])
                            nc.sync.dma_start(dst, obt[:])
                        return
                    if co % 2 == 1 or (drain and co < 2):
                        nc.scalar.copy(ob[:, gi, :], ps_p[:])
                    else:
                        nc.vector.tensor_copy(ob[:, gi, :], ps_p[:])
                    if dma_n:
                        dst = oT_part[128 * (co + 1 - dma_n):128 * (co + 1),
                                      TB * j:TB * (j + 1)]
                        q = nc.scalar if (co // 2) % 2 else nc.sync
                        q.dma_start(
                            dst.rearrange("(u p) t -> p u t", p=128),
                            ob[:, gi + 1 - dma_n:gi + 1, :])
                oproj_unit.ob = None

                def oproj_half(at_j, co):
                    """Heads-(0,1) half of a tail-block o_proj chunk, staged
                    to SBUF so only 2 matmuls + a combine remain at the end."""
                    ps_p = psB.tile([128, TB], f32, tag="ps_d", bufs=2,
                                    name="ps_h")
                    for jh in range(2):
                        nc.tensor.matmul(ps_p[:],
                                         wo_all[:, jh * HID + 128 * co:
                                                jh * HID + 128 * (co + 1)],
                                         at_j[jh][:], start=(jh == 0),
                                         stop=(jh == 1))
                    hob = pB.tile([128, TB], f32, tag="hob", bufs=16,
                                  name="hob")
                    if co % 2:
                        nc.scalar.copy(hob[:], ps_p[:])
                    else:
                        nc.vector.tensor_copy(hob[:], ps_p[:])
                    return hob

                def oproj_finish(j, at_j, co, hob, dma_n):
                    """Tail-block finish: heads (2,3) + combine with the
                    staged half. Combines alternate DVE/Pool (both idle at
                    the drain), DMAs as in oproj_unit."""
                    gi = co % 2
                    ob = oproj_unit.ob
                    if gi == 0 and dma_n != 1:
                        ob = oproj_unit.ob = pB.tile([128, 2, TB], f32,
                                                     tag="ob", bufs=4,
                                                     name="ob")
                    ps_p = psB.tile([128, TB], f32, tag="ps_d", bufs=2,
                                    name="ps_f")
                    for jh in (2, 3):
                        nc.tensor.matmul(ps_p[:],
                                         wo_all[:, jh * HID + 128 * co:
                                                jh * HID + 128 * (co + 1)],
                                         at_j[jh][:], start=(jh == 2),
                                         stop=(jh == 3))
                    eng = nc.vector if co % 2 == 0 else nc.gpsimd
                    if dma_n == 1:
                        if co < 4 and not with_rs:
                            dst = out_r[128 * co:128 * (co + 1),
                                        TB * j:TB * (j + 1)]
                            obt = pB.tile([128, TB], f32, tag="obt", bufs=2,
                                          name="obt")
                        else:
                            dst = oT_part[128 * co:128 * (co + 1),
                                          TB * j:TB * (j + 1)]
                            obt = pB.tile([128, TB], f32, tag="obtb", bufs=2,
                                          name="obtb")
                        eng.tensor_tensor(obt[:], hob[:], ps_p[:], op=ADD)
                        q = nc.scalar if co == NCT - 1 else nc.sync
                        q.dma_start(dst, obt[:])
                        return
                    eng.tensor_tensor(ob[:, gi, :], hob[:], ps_p[:], op=ADD)
                    if dma_n:
                        dst = oT_part[128 * (co + 1 - dma_n):128 * (co + 1),
                                      TB * j:TB * (j + 1)]
                        q = nc.scalar if (co // 2) % 2 else nc.sync
                        q.dma_start(
                            dst.rearrange("(u p) t -> p u t", p=128),
                            ob[:, gi + 1 - dma_n:gi + 1, :])

                def make_units(j, at_j, tail=False):
                    # groups of 2 co per DMA into bf16 oT_part; single-co DMAs
                    # for the out_r slice (no-RS build) and the tail block's
                    # last four co (short final transfers)
                    out = []
                    for co in range(NCT):
                        if (co < 4 and not with_rs) or (tail and co >= NCT - 4):
                            dma_n = 1
                        else:
                            dma_n = 2 if co % 2 == 1 else 0
                        out.append((lambda jj, aa, cc, dd:
                                    lambda drain=False: oproj_unit(
                                        jj, aa, cc, dd, drain))
                                   (j, at_j, co, dma_n))
                    return out

                order = [1, 2, 3, 0]
                for bi, j in enumerate(order):
                    last = bi == len(order) - 1
                    if causal:
                        tiles = [(i, 0) for i in range(4 * j)]
                        tiles += [(4 * j + m, min(128 * m, 256))
                                  for m in range(4)]
                    else:
                        tiles = [(i, 0) for i in range(NKT)]
                    last_i = tiles[-1][0]
                    n_iter = 2 * len(tiles)
                    # interleave cadence: previous block's units spread over
                    # this block's tile stream, holding 2 back per pass end
                    # (they cover the denominator-reciprocal PSUM-slot WAR)
                    res_units = units[max(0, len(units) - 4):]
                    units = units[:max(0, len(units) - 4)]
                    spread = len(units)
                    credit = 0.0
                    halves, halves_todo = [], []

                    at_j = [pB.tile([128, TB], f32r, tag=f"at{h}", bufs=2,
                                    name=f"at_s{h}") for h in range(GH)]
                    for p in range(2):
                        h0, h1 = 2 * p, 2 * p + 1
                        acc = pB.tile([128, 2 * TB], bf16, tag="accp", bufs=2,
                                      name="acc")
                        ps_o = {h: psB.tile([128, TB], f32, tag="po", bufs=2,
                                            name=f"ps_o{h}") for h in (h0, h1)}
                        for ti, (i, off) in enumerate(tiles):
                            w = TB - off
                            diag = causal and i >= 4 * j
                            m = i - 4 * j if diag else -1
                            ps2 = psB.tile([128, 2 * TB], f32, tag="ps_s",
                                           bufs=2, name="ps2")
                            for hh, h in enumerate((h0, h1)):
                                nc.tensor.matmul(
                                    ps2[:, TB * hh:TB * hh + w],
                                    kt_rope[:, 128 * i:128 * (i + 1)],
                                    qt_rope[h][:, TB * j + off:TB * (j + 1)],
                                    start=True, stop=True)
                            pt2 = pB.tile([128, 2 * TB], bf16, tag="pt",
                                          bufs=8, name="pt")
                            if w == TB:
                                nc.scalar.activation(pt2[:], ps2[:], AF.Exp)
                            else:
                                pr = pt2[:].rearrange("p (u q) -> p u q", u=2)
                                sr = ps2[:].rearrange("p (u q) -> p u q", u=2)
                                nc.scalar.activation(pr[:, :, 0:w],
                                                     sr[:, :, 0:w], AF.Exp)
                            if diag:
                                patt, pw = (1024, 256) if m == 3 else (0, TB)
                                cr = (cm_b[:, patt:patt + 2 * pw]
                                      .rearrange("p (u q) -> p u q", u=2))
                                pr = pt2[:].rearrange("p (u q) -> p u q", u=2)
                                nc.vector.tensor_tensor(
                                    pr[:, :, 0:w], pr[:, :, 0:w],
                                    cr[:, :, 0:w], op=MUL)
                            if ti == 0:
                                nc.vector.tensor_copy(acc[:], pt2[:])
                            else:
                                ar = acc[:].rearrange("p (u q) -> p u q", u=2)
                                pr = pt2[:].rearrange("p (u q) -> p u q", u=2)
                                nc.vector.tensor_tensor(
                                    ar[:, :, off:TB], ar[:, :, off:TB],
                                    pr[:, :, 0:w], op=ADD)
                            # PV (m=3 keep region is only the last 128 cols)
                            pv_off = 384 if m == 3 else off
                            for hh, h in enumerate((h0, h1)):
                                nc.tensor.matmul(
                                    ps_o[h][:, pv_off:TB],
                                    v_all[:, 128 * i:128 * (i + 1)],
                                    pt2[:, TB * hh + pv_off - off:
                                        TB * hh + TB - off],
                                    start=(ti == 0), stop=(i == last_i),
                                    skip_group_check=True)
                            if last:
                                # tail block: previous block's units in pass
                                # 0, own heads-(0,1) o_proj halves in pass 1
                                credit += (spread if p == 0 else 16.0) \
                                    / len(tiles)
                            else:
                                credit += spread / n_iter
                            while credit >= 1.0 and (units or halves_todo):
                                credit -= 1.0
                                if units:
                                    units.pop(0)()
                                else:
                                    co_h = halves_todo.pop(0)
                                    halves.append((co_h,
                                                   oproj_half(at_j, co_h)))
                        # pass end: leftover halves, then reserved units (they
                        # fill the PE while the normalize chain runs), then
                        # denominators into the ps_d ring so the score-PSUM
                        # ring never waits on the reciprocal
                        if last and p == 1:
                            while halves_todo:
                                co_h = halves_todo.pop(0)
                                halves.append((co_h, oproj_half(at_j, co_h)))
                        for _ in range(min(2, len(res_units))):
                            res_units.pop(0)()
                        for hh, h in enumerate((h0, h1)):
                            pd = psB.tile([128, TB], f32, tag="ps_d", bufs=2,
                                          name="pd")
                            nc.tensor.matmul(pd[:], ones_b[:],
                                             acc[:, TB * hh:TB * (hh + 1)],
                                             start=True, stop=True)
                            rec = pB.tile([128, TB], f32, tag="rec", bufs=4,
                                          name="rec")
                            nc.vector.reciprocal(rec[:], pd[:])
                            nc.vector.tensor_tensor(at_j[h][:], ps_o[h][:],
                                                    rec[:], op=MUL)
                        if last and p == 0:
                            halves_todo = list(range(NCT))
                    for u in units + res_units:
                        u(True)
                    if last:
                        units = []
                        for co, hob in halves:
                            if (co < 4 and not with_rs) or co >= NCT - 4:
                                dn = 1
                            else:
                                dn = 2 if co % 2 == 1 else 0
                            oproj_finish(j, at_j, co, hob, dn)
                    else:
                        units = make_units(j, at_j)

            # ---- phase C: ReduceScatter partials, emit this core's slice ----
            if with_rs:
                nc.gpsimd.collective_compute(
                    "ReduceScatter", ADD,
                    replica_groups=[[0, 1, 2, 3], [4, 5, 6, 7]],
                    ins=[oT_part[:].opt()], outs=[oT_red[:].opt()],
                )
                nc.sync.dma_start(out_r[:], oT_red[:])

    nc.compile()
    return nc


def kernel(hidden_states, attention_mask, Wq, Wk, Wv, Wo, sin, cos):
    hidden_states = np.asarray(hidden_states, dtype=np.float32)
    attention_mask = np.asarray(attention_mask, dtype=np.float32)
    Wq, Wk, Wv, Wo = (np.ascontiguousarray(np.asarray(a, dtype=np.float32))
                      for a in (Wq, Wk, Wv, Wo))
    sin = np.asarray(sin, dtype=np.float32)
    cos = np.asarray(cos, dtype=np.float32)

    # classify the mask: causal (top-right strictly very-negative, elsewhere 0,
    # col 0 ignored since reference zeroes it) vs all-zeros (full attention)
    m0 = attention_mask[0, 0]
    iu = np.triu_indices(S, k=1)
    causal = bool((m0[iu] < -1e30).all() and
                  (m0[np.tril_indices(S, k=0)] == 0.0).all())
    if not causal:
        assert (attention_mask == 0).all(), "unsupported attention mask pattern"
    if causal:
        for b in range(1, B):
            assert np.array_equal(attention_mask[b, 0], m0), "mask differs per batch"

    key = causal
    if key not in _CACHE:
        _CACHE[key] = _build(causal)
    nc = _CACHE[key]

    import ml_dtypes
    nbf16 = ml_dtypes.bfloat16
    cos_t = np.ascontiguousarray(cos[:S].T)          # [128, S]
    sin_m = np.ascontiguousarray(sin[:S].T)
    sin_m[:64] *= -1.0
    # paired 0/1 causal keep-patterns (each repeated twice for head pairs):
    # patt0 = (q >= k) at cols 0:1024, patt1 = (q >= k + 128) at cols 1024:1536
    kl = np.arange(128)[:, None]
    ql = np.arange(512)[None, :]
    p0 = (ql >= kl).astype(np.float32)
    p1 = (ql[:, :256] >= kl + 128).astype(np.float32)
    cmask = np.concatenate([p0, p0, p1, p1], axis=1).astype(nbf16)

    in_maps = []
    for c in range(8):
        b, g = c // 4, c % 4
        in_maps.append({
            "xt": np.ascontiguousarray(hidden_states[b].T),
            "wq": np.ascontiguousarray(Wq[512 * g:512 * (g + 1), :].T),
            "wk": np.ascontiguousarray(Wk[128 * g:128 * (g + 1), :].T),
            "wv": np.ascontiguousarray(Wv[128 * g:128 * (g + 1), :].T),
            "wo": np.ascontiguousarray(Wo[:, 512 * g:512 * (g + 1)].T),
            "cos_t": cos_t, "sin_m": sin_m, "cmask": cmask,
            "ones_in": np.ones((128, 128), dtype=nbf16),
            "ident_in": np.eye(128, dtype=np.float32).astype(nbf16),
        })

    global _LAST_IN_MAPS, _LAST_RES
    _LAST_IN_MAPS = in_maps
    res = run_bass_kernel_spmd(nc, in_maps, core_ids=list(range(8)))
    _LAST_RES = res

    out = np.empty((B, S, HID), dtype=np.float32)
    for c in range(8):
        b, r = c // 4, c % 4
        out[b, :, TB * r:TB * (r + 1)] = res.results[c]["out_r"].T
    return out


if __name__ == "__main__":
    print("module loads ok")


# revision 50
# speedup vs baseline: 1.0538x; 1.0538x over previous
"""HF OpenMoe attention (B=2,S=2048,HID=2048,NH=16,NKV=4,HD=128) on 8 trn2 cores.

Sharding: core c -> (batch b=c//4, kv-group g=c%4). Each core computes Q/K/V
projections for its 4 query heads + 1 kv head, RoPE, causal flash attention in
S^T layout (scores transposed: [k, q], softmax over the partition dim via
ones-matmul), and its partial o_proj; a 4-way ReduceScatter sums the o_proj
partials, each core returning a 512-row slice of o^T for its batch.

Phase A streams x^T per token block with weights interleaved into the DMA
stream (wk chunked between xt tiles, wv mid-stream, wq after block 0), PE
chains emitted as operands land, rope on DVE/Act/Pool. Phase B runs causal
attention per q-block in two 2-head passes: score pairs land in a 2-bank PSUM
tile, one exp covers both heads, mask/acc run as paired 2x DVE ops, the
paired denominator reuses the score-PSUM ring, and o_proj of the previous
block is emitted in 1-co units between score tiles so the PE keeps running
through the Act-paced exp stream. The final block (j=0, shortest) holds a few
units in reserve to cover its normalize latency; tail o_proj DMAs are split
small and output DMAs ride the Act queue.
"""
import numpy as np
import concourse.bass as bass
import concourse.bacc as bacc
import concourse.tile as tile
import concourse.mybir as mybir
from concourse.bass_utils import run_bass_kernel_spmd

f32 = mybir.dt.float32
f32r = mybir.dt.float32r
bf16 = mybir.dt.bfloat16
AF = mybir.ActivationFunctionType
MUL = mybir.AluOpType.mult
ADD = mybir.AluOpType.add

B, S, HID = 2, 2048, 2048
NH, NKV, HD = 16, 4, 128
GH = NH // NKV          # query heads per core (4)
TB = 512                # token block (q block / projection block)
NT = S // TB            # 4 token blocks
NCT = HID // 128        # 16 contraction tiles
NKT = S // 128          # 16 key tiles

_CACHE = {}


def _build(causal: bool, with_rs: bool = True):
    nc = bacc.Bacc("TRN2", target_bir_lowering=False, debug=False, num_devices=8)
    xt = nc.dram_tensor("xt", [HID, S], f32, kind="ExternalInput").ap()
    wq = nc.dram_tensor("wq", [HID, GH * HD], f32, kind="ExternalInput").ap()
    wk = nc.dram_tensor("wk", [HID, HD], f32, kind="ExternalInput").ap()
    wv = nc.dram_tensor("wv", [HID, HD], f32, kind="ExternalInput").ap()
    wo = nc.dram_tensor("wo", [GH * HD, HID], f32, kind="ExternalInput").ap()
    cos_d = nc.dram_tensor("cos_t", [HD, S], f32, kind="ExternalInput").ap()
    sin_d = nc.dram_tensor("sin_m", [HD, S], f32, kind="ExternalInput").ap()
    cm_d = nc.dram_tensor("cmask", [128, 1536], bf16, kind="ExternalInput").ap()
    on_d = nc.dram_tensor("ones_in", [128, 128], bf16, kind="ExternalInput").ap()
    id_d = nc.dram_tensor("ident_in", [128, 128], bf16, kind="ExternalInput").ap()
    out_r = nc.dram_tensor("out_r", [TB, S], f32, kind="ExternalOutput").ap()

    with tile.TileContext(nc) as tc:
        with (
            tc.tile_pool(name="glob", bufs=1) as glob,
            tc.tile_pool(name="dram", bufs=1, space="DRAM") as dram,
        ):
            # ---- global resident stores ----
            kt_rope = glob.tile([128, S], f32r, tag="kt")          # roped K^T [d, k]
            v_all = glob.tile([128, S], bf16, tag="v")             # V natural (bf16)
            qt_rope = [glob.tile([128, S], f32r, tag=f"q{h}", name=f"qt_rope{h}")
                       for h in range(GH)]
            cm_b = glob.tile([128, 1536], bf16, tag="cmb")         # paired 0/1 masks
            ones_b = glob.tile([128, 128], bf16, tag="onesb")
            ident_b = glob.tile([128, 128], bf16, tag="identb")

            oT_part = dram.tile([HID, S], f32)                     # o^T partial
            oT_red = dram.tile([TB, S], f32)

            # ---- phase A: projections + rope (phase-scoped SBUF) ----
            with tc.tile_pool(name="pA", bufs=1) as pA, \
                 tc.tile_pool(name="psA", bufs=1, space="PSUM") as psA:
                # batched weight loads: DRAM [c, d] -> SBUF [c-sub(128), ci, d].
                # Issue order matters: the DMA engines drain in order, so wk
                # comes first (chunked between xt tiles), wv/wq mid-stream,
                # and wo not until phase B.
                wk_all = pA.tile([128, NCT, HD], f32r, tag="wk")
                nc.sync.dma_start(wk_all[:, 0:1, :], wk[0:128, :].bitcast(f32r)
                                  .rearrange("(c p) d -> p c d", p=128))
                wv_all = pA.tile([128, NCT, HD], f32r, tag="wv")
                wq_all = [pA.tile([128, NCT, HD], f32r, tag=f"wqh{h}",
                                  name=f"wq_all{h}") for h in range(GH)]

                def rope(ps, dst_ap, cs, sn):
                    """dst = ps*cos + swap64(ps)*sin_mod for token block tb."""
                    raw = pA.tile([128, TB], f32, tag="raw", bufs=3, name="raw")
                    nc.scalar.copy(raw[:], ps[:])
                    rot = pA.tile([128, TB], f32, tag="rot", bufs=6, name="rot")
                    nc.gpsimd.dma_start(rot[0:64, :], raw[64:128, :])
                    nc.gpsimd.dma_start(rot[64:128, :], raw[0:64, :])
                    m1 = pA.tile([128, TB], f32, tag="m1", bufs=6, name="m1")
                    nc.vector.tensor_tensor(m1[:], ps[:], cs[:], op=MUL)  # PSUM: DVE
                    nc.vector.tensor_tensor(rot[:], rot[:], sn[:], op=MUL)
                    nc.vector.tensor_tensor(dst_ap, m1[:], rot[:], op=ADD)

                for tb in range(NT):
                    cos_s = pA.tile([128, TB], f32, tag="cos", bufs=4, name="cos")
                    sin_s = pA.tile([128, TB], f32, tag="sin", bufs=4, name="sin")
                    if tb != 0:
                        # rope tables just ahead of the xt tiles
                        nc.sync.dma_start(cos_s[:], cos_d[:, TB * tb:TB * (tb + 1)])
                        nc.sync.dma_start(sin_s[:], sin_d[:, TB * tb:TB * (tb + 1)])
                    xt_t = []
                    for ci in range(NCT):
                        t = pA.tile([128, TB], f32r, tag="xt", bufs=31, name="xt")
                        nc.sync.dma_start(
                            t[:], xt[128 * ci:128 * (ci + 1),
                                     TB * tb:TB * (tb + 1)].bitcast(f32r))
                        xt_t.append(t)
                        if tb == 0 and ci == 0:
                            nc.sync.dma_start(
                                wk_all[:, 1:4, :], wk[128:512, :].bitcast(f32r)
                                .rearrange("(c p) d -> p c d", p=128))
                        if tb == 0 and ci % 4 == 3 and ci < 15:
                            c = ci // 4 + 1  # stream wk in behind the xt tiles
                            nc.sync.dma_start(
                                wk_all[:, 4 * c:4 * (c + 1), :],
                                wk[512 * c:512 * (c + 1), :].bitcast(f32r)
                                .rearrange("(c p) d -> p c d", p=128))
                        if tb == 0 and ci == 0:
                            nc.sync.dma_start(cos_s[:], cos_d[:, 0:TB])
                            nc.sync.dma_start(sin_s[:], sin_d[:, 0:TB])
                        if tb == 0 and ci == 1:
                            nc.sync.dma_start(ident_b[:], id_d[:])
                            warm = pA.tile([128, 1], f32, tag="warm")
                            nc.scalar.activation(warm[:], ident_b[:, 0:1], AF.Exp)
                        if tb == 0 and ci == 6:
                            # wv early: V proj fills the K-chain's DMA lag
                            nc.sync.dma_start(wv_all[:], wv[:].bitcast(f32r)
                                              .rearrange("(c p) d -> p c d", p=128))
                        if tb == 0 and ci == 10:
                            # first q head's weights ahead of the rest
                            nc.sync.dma_start(
                                wq_all[0][:], wq[:, 0:HD]
                                .bitcast(f32r).rearrange("(c p) d -> p c d", p=128))
                    if tb == 0:
                        for h in range(1, GH):
                            nc.sync.dma_start(
                                wq_all[h][:], wq[:, HD * h:HD * (h + 1)]
                                .bitcast(f32r).rearrange("(c p) d -> p c d", p=128))
                    # K
                    ps_k = psA.tile([128, TB], f32, tag="pk")
                    for ci in range(NCT):
                        nc.tensor.matmul(ps_k[:], wk_all[:, ci, :], xt_t[ci][:],
                                         start=ci == 0, stop=ci == NCT - 1)
                    rope(ps_k, kt_rope[:, TB * tb:TB * (tb + 1)], cos_s, sin_s)

                    def emit_v():
                        ps_v = psA.tile([128, TB], f32, tag="pv")
                        for ci in range(NCT):
                            nc.tensor.matmul(ps_v[:], wv_all[:, ci, :],
                                             xt_t[ci][:],
                                             start=ci == 0, stop=ci == NCT - 1)
                        vt_sb = pA.tile([128, TB], bf16, tag="vts", bufs=3,
                                        name="vt_sb")
                        nc.scalar.copy(vt_sb[:], ps_v[:])
                        return vt_sb

                    def emit_vtrans(vt_sb, u):
                        ps_tr = psA.tile([128, 128], bf16, tag="ptr", bufs=1,
                                         name="ps_tr")
                        nc.tensor.transpose(ps_tr[:],
                                            vt_sb[:, 128 * u:128 * (u + 1)],
                                            ident_b[:])
                        # Act (not DVE): DVE's in-order queue sits behind rope
                        # m1 ops that can wait on the cos/sin loads
                        nc.scalar.copy(
                            v_all[:, 128 * (4 * tb + u):128 * (4 * tb + u + 1)],
                            ps_tr[:])

                    vt_sb = emit_v() if tb < NT - 1 else None
                    # Q heads; one V transpose is spread between each pair of
                    # head blocks so the single ptr bank's WAR (on the previous
                    # transpose's drain copy) never stalls the PE
                    for h in range(GH):
                        ps_q = psA.tile([128, TB], f32, tag="pq", bufs=5,
                                        name=f"ps_q{h}")
                        for ci in range(NCT):
                            nc.tensor.matmul(ps_q[:], wq_all[h][:, ci, :],
                                             xt_t[ci][:],
                                             start=ci == 0, stop=ci == NCT - 1)
                        if vt_sb is not None:
                            emit_vtrans(vt_sb, h)
                        rope(ps_q, qt_rope[h][:, TB * tb:TB * (tb + 1)], cos_s,
                             sin_s)
                    if vt_sb is None:
                        # last block: V after the Q heads, hiding the final
                        # rope chain's latency behind V's matmuls
                        vt_sb = emit_v()
                        for u in range(4):
                            emit_vtrans(vt_sb, u)

            # ---- phase B: attention (2-head passes) + partial o_proj ----
            with tc.tile_pool(name="pB", bufs=1) as pB, \
                 tc.tile_pool(name="psB", bufs=1, space="PSUM") as psB:
                # small constants are bf16 in DRAM: direct loads, no casts
                nc.sync.dma_start(cm_b[:], cm_d[:])
                nc.sync.dma_start(ones_b[:], on_d[:])
                # o_proj weights: first o_proj unit runs well into phase B, so
                # this load hides behind the first attention block
                wo_all = pB.tile([128, GH * HID], f32r, tag="wo")  # [j-sub, jh*2048+c]
                nc.sync.dma_start(wo_all[:].rearrange("p (h c) -> p h c", h=GH),
                                  wo[:].bitcast(f32r)
                                  .rearrange("(h p) c -> p h c", p=128))

                units = []   # pending o_proj 1-co unit closures

                def oproj_unit(j, at_j, co, dma_n, drain=False):
                    """One 128-col chunk of q-block j's o_proj. dma_n=2: flush
                    the 2-co ob group; dma_n=1: single-co DMA. In drain mode
                    (no tile stream running) copies favor Act and DMAs spread
                    over the idle sync/gpsimd queues."""
                    gi = co % 2
                    ob = oproj_unit.ob
                    if gi == 0 and dma_n != 1:
                        ob = oproj_unit.ob = pB.tile([128, 2, TB], f32,
                                                     tag="ob", bufs=4,
                                                     name="ob")
                    ps_p = psB.tile([128, TB], f32, tag="ps_d", bufs=2,
                                    name="ps_p")
                    for jh in range(GH):
                        nc.tensor.matmul(ps_p[:],
                                         wo_all[:, jh * HID + 128 * co:
                                                jh * HID + 128 * (co + 1)],
                                         at_j[jh][:], start=(jh == 0),
                                         stop=(jh == GH - 1))
                    if dma_n == 1:
                        if co < 4 and not with_rs:
                            dst = out_r[128 * co:128 * (co + 1),
                                        TB * j:TB * (j + 1)]
                            obt = pB.tile([128, TB], f32, tag="obt", bufs=2,
                                          name="obt")
                        else:
                            dst = oT_part[128 * co:128 * (co + 1),
                                          TB * j:TB * (j + 1)]
                            obt = pB.tile([128, TB], f32, tag="obtb", bufs=2,
                                          name="obtb")
                        if co % 2:
                            nc.scalar.copy(obt[:], ps_p[:])
                            # keep the Act queue free of DMA issues ahead of
                            # the final copy: only the very last co rides Act
                            q = nc.scalar if co == NCT - 1 else nc.sync
                            q.dma_start(dst, obt[:])
                        else:
                            nc.vector.tensor_copy(obt[:], ps_p[:])
                            nc.sync.dma_start(dst, obt[:])
                        return
                    if co % 2 == 1 or (drain and co < 2):
                        nc.scalar.copy(ob[:, gi, :], ps_p[:])
                    else:
                        nc.vector.tensor_copy(ob[:, gi, :], ps_p[:])
                    if dma_n:
                        dst = oT_part[128 * (co + 1 - dma_n):128 * (co + 1),
                                      TB * j:TB * (j + 1)]
                        q = nc.scalar if (co // 2) % 2 else nc.sync
                        q.dma_start(
                            dst.rearrange("(u p) t -> p u t", p=128),
                            ob[:, gi + 1 - dma_n:gi + 1, :])
                oproj_unit.ob = None

                def make_units(j, at_j, tail=False):
                    # groups of 2 co per DMA into bf16 oT_part; single-co DMAs
                    # for the out_r slice (no-RS build) and the tail block's
                    # last four co (short final transfers)
                    out = []
                    for co in range(NCT):
                        if (co < 4 and not with_rs) or (tail and co >= NCT - 4):
                            dma_n = 1
                        else:
                            dma_n = 2 if co % 2 == 1 else 0
                        out.append((lambda jj, aa, cc, dd:
                                    lambda drain=False: oproj_unit(
                                        jj, aa, cc, dd, drain))
                                   (j, at_j, co, dma_n))
                    return out

                order = [1, 2, 3, 0]
                for bi, j in enumerate(order):
                    last = bi == len(order) - 1
                    if causal:
                        tiles = [(i, 0) for i in range(4 * j)]
                        tiles += [(4 * j + m, min(128 * m, 256))
                                  for m in range(4)]
                    else:
                        tiles = [(i, 0) for i in range(NKT)]
                    last_i = tiles[-1][0]
                    n_iter = 2 * len(tiles)
                    # interleave cadence: previous block's units spread over
                    # this block's tile stream, holding 2 back per pass end
                    # (they cover the denominator-reciprocal PSUM-slot WAR)
                    res_units = units[max(0, len(units) - 4):]
                    units = units[:max(0, len(units) - 4)]
                    spread = len(units)
                    credit = 0.0

                    at_j = [pB.tile([128, TB], f32r, tag=f"at{h}", bufs=2,
                                    name=f"at_s{h}") for h in range(GH)]
                    for p in range(2):
                        h0, h1 = 2 * p, 2 * p + 1
                        acc = pB.tile([128, 2 * TB], bf16, tag="accp", bufs=2,
                                      name="acc")
                        ps_o = {h: psB.tile([128, TB], f32, tag="po", bufs=2,
                                            name=f"ps_o{h}") for h in (h0, h1)}
                        for ti, (i, off) in enumerate(tiles):
                            w = TB - off
                            diag = causal and i >= 4 * j
                            m = i - 4 * j if diag else -1
                            ps2 = psB.tile([128, 2 * TB], f32, tag="ps_s",
                                           bufs=2, name="ps2")
                            for hh, h in enumerate((h0, h1)):
                                nc.tensor.matmul(
                                    ps2[:, TB * hh:TB * hh + w],
                                    kt_rope[:, 128 * i:128 * (i + 1)],
                                    qt_rope[h][:, TB * j + off:TB * (j + 1)],
                                    start=True, stop=True)
                            pt2 = pB.tile([128, 2 * TB], bf16, tag="pt",
                                          bufs=8, name="pt")
                            if w == TB:
                                nc.scalar.activation(pt2[:], ps2[:], AF.Exp)
                            else:
                                pr = pt2[:].rearrange("p (u q) -> p u q", u=2)
                                sr = ps2[:].rearrange("p (u q) -> p u q", u=2)
                                nc.scalar.activation(pr[:, :, 0:w],
                                                     sr[:, :, 0:w], AF.Exp)
                            if diag:
                                patt, pw = (1024, 256) if m == 3 else (0, TB)
                                cr = (cm_b[:, patt:patt + 2 * pw]
                                      .rearrange("p (u q) -> p u q", u=2))
                                pr = pt2[:].rearrange("p (u q) -> p u q", u=2)
                                nc.vector.tensor_tensor(
                                    pr[:, :, 0:w], pr[:, :, 0:w],
                                    cr[:, :, 0:w], op=MUL)
                            if ti == 0:
                                nc.vector.tensor_copy(acc[:], pt2[:])
                            else:
                                ar = acc[:].rearrange("p (u q) -> p u q", u=2)
                                pr = pt2[:].rearrange("p (u q) -> p u q", u=2)
                                nc.vector.tensor_tensor(
                                    ar[:, :, off:TB], ar[:, :, off:TB],
                                    pr[:, :, 0:w], op=ADD)
                            # PV (m=3 keep region is only the last 128 cols)
                            pv_off = 384 if m == 3 else off
                            for hh, h in enumerate((h0, h1)):
                                nc.tensor.matmul(
                                    ps_o[h][:, pv_off:TB],
                                    v_all[:, 128 * i:128 * (i + 1)],
                                    pt2[:, TB * hh + pv_off - off:
                                        TB * hh + TB - off],
                                    start=(ti == 0), stop=(i == last_i),
                                    skip_group_check=True)
                            credit += spread / n_iter
                            while credit >= 1.0 and units:
                                credit -= 1.0
                                units.pop(0)()
                        # pass end: reserved units first (they fill the PE
                        # while the normalize chain runs), then denominators
                        # into the ps_d ring so the score-PSUM ring never
                        # waits on the reciprocal
                        for _ in range(min(2, len(res_units))):
                            res_units.pop(0)()
                        for hh, h in enumerate((h0, h1)):
                            pd = psB.tile([128, TB], f32, tag="ps_d", bufs=2,
                                          name="pd")
                            nc.tensor.matmul(pd[:], ones_b[:],
                                             acc[:, TB * hh:TB * (hh + 1)],
                                             start=True, stop=True)
                            rec = pB.tile([128, TB], f32, tag="rec", bufs=4,
                                          name="rec")
                            nc.vector.reciprocal(rec[:], pd[:])
                            nc.vector.tensor_tensor(at_j[h][:], ps_o[h][:],
                                                    rec[:], op=MUL)
                    for u in units + res_units:
                        u(True)
                    units = make_units(j, at_j, tail=last)
                for u in units:
                    u(True)

            # ---- phase C: ReduceScatter partials, emit this core's slice ----
            if with_rs:
                nc.gpsimd.collective_compute(
                    "ReduceScatter", ADD,
                    replica_groups=[[0, 1, 2, 3], [4, 5, 6, 7]],
                    ins=[oT_part[:].opt()], outs=[oT_red[:].opt()],
                )
                nc.sync.dma_start(out_r[:], oT_red[:])

    nc.compile()
    return nc


def kernel(hidden_states, attention_mask, Wq, Wk, Wv, Wo, sin, cos):
    hidden_states = np.asarray(hidden_states, dtype=np.float32)
    attention_mask = np.asarray(attention_mask, dtype=np.float32)
    Wq, Wk, Wv, Wo = (np.ascontiguousarray(np.asarray(a, dtype=np.float32))
                      for a in (Wq, Wk, Wv, Wo))
    sin = np.asarray(sin, dtype=np.float32)
    cos = np.asarray(cos, dtype=np.float32)

    # classify the mask: causal (top-right strictly very-negative, elsewhere 0,
    # col 0 ignored since reference zeroes it) vs all-zeros (full attention)
    m0 = attention_mask[0, 0]
    iu = np.triu_indices(S, k=1)
    causal = bool((m0[iu] < -1e30).all() and
                  (m0[np.tril_indices(S, k=0)] == 0.0).all())
    if not causal:
        assert (attention_mask == 0).all(), "unsupported attention mask pattern"
    if causal:
        for b in range(1, B):
            assert np.array_equal(attention_mask[b, 0], m0), "mask differs per batch"

    key = causal
    if key not in _CACHE:
        _CACHE[key] = _build(causal)
    nc = _CACHE[key]

    import ml_dtypes
    nbf16 = ml_dtypes.bfloat16
    cos_t = np.ascontiguousarray(cos[:S].T)          # [128, S]
    sin_m = np.ascontiguousarray(sin[:S].T)
    sin_m[:64] *= -1.0
    # paired 0/1 causal keep-patterns (each repeated twice for head pairs):
    # patt0 = (q >= k) at cols 0:1024, patt1 = (q >= k + 128) at cols 1024:1536
    kl = np.arange(128)[:, None]
    ql = np.arange(512)[None, :]
    p0 = (ql >= kl).astype(np.float32)
    p1 = (ql[:, :256] >= kl + 128).astype(np.float32)
    cmask = np.concatenate([p0, p0, p1, p1], axis=1).astype(nbf16)

    in_maps = []
    for c in range(8):
        b, g = c // 4, c % 4
        in_maps.append({
            "xt": np.ascontiguousarray(hidden_states[b].T),
            "wq": np.ascontiguousarray(Wq[512 * g:512 * (g + 1), :].T),
            "wk": np.ascontiguousarray(Wk[128 * g:128 * (g + 1), :].T),
            "wv": np.ascontiguousarray(Wv[128 * g:128 * (g + 1), :].T),
            "wo": np.ascontiguousarray(Wo[:, 512 * g:512 * (g + 1)].T),
            "cos_t": cos_t, "sin_m": sin_m, "cmask": cmask,
            "ones_in": np.ones((128, 128), dtype=nbf16),
            "ident_in": np.eye(128, dtype=np.float32).astype(nbf16),
        })

    global _LAST_IN_MAPS, _LAST_RES
    _LAST_IN_MAPS = in_maps
    res = run_bass_kernel_spmd(nc, in_maps, core_ids=list(range(8)))
    _LAST_RES = res

    out = np.empty((B, S, HID), dtype=np.float32)
    for c in range(8):
        b, r = c // 4, c % 4
        out[b, :, TB * r:TB * (r + 1)] = res.results[c]["out_r"].T
    return out


if __name__ == "__main__":
    print("module loads ok")
